# revision 69
# baseline (speedup 1.0000x reference)
"""Trainium2 Bass kernel for CardAwarePolicy (counts-reformulated MHA + folded MLPs).

Self-contained: takes full unsharded inputs, shards batch across 8 NeuronCores
(pure data parallel), runs a Tile/Bass kernel per core, gathers the output.
~103-105us HW exec vs the 122.6us f32r baseline; norm rel err ~5.9e-3.

Design notes:
  - Math: the masked 4-head self-attention over hand slots depends on the
    hand only through its card-count vector, so softmax folds into fixed
    [54x54] exp tables (den -> 1/den -> T -> w2 -> Y chain); all MLPs are
    pre-folded into single matmuls on host.
  - bf16 weights/activations everywhere except: the w2 matmuls (f32r, rhs
    comes from the f32-only gpsimd T-mult) and the f32 reciprocal
    (reciprocal_approx_fast requires f32). bf16 LDWEIGHTS needs >=
    multiple-of-32 partition counts -> den tables padded 54 -> 64 rows.
  - Software-pipelined emission: stage A (den/recip/T) runs one tile ahead
    of stage B (w2 onward) so the in-order PE always has queued work while
    the recip->T chain of the current tile completes.
  - Engine split per tile: DVE recip + 2 Y-mults + 3 H-relus (bf16 SBUF 4x
    mode); Act gd1r/ctx1 relus, u4 copy, 2 H-relus, output bias; GpSimd one
    f32 T-mult (keeps DVE under the PE's ~3us/tile). Large DMA triggers on
    the sync sequencer (small nscb before the big nsct so tile 0 starts
    early); weight blobs trigger via Act. The last three tiles route T and
    all H-relus to the DVE, which is idle during pipeline drain. Emission
    order per tile is B(t), A(t+2), C(t): the data-ready den matmuls of
    t+2 keep the in-order PE fed while C(t)'s score matmuls wait on the
    H-relu latency.
  - PSUM (8 banks): att tag = den/w2 [108,1024] 2-bank tiles (bufs=2), g1
    tag = gd1/ctx1/u4 rotation (bufs=2), sc4 tag double-buffered.
  - Scores for 3 tiles accumulate into one PSUM bank at 32-partition
    offsets (quadrant 3 is unusable) after a zero-weight matmul pre-clears
    the bank's has_written bits; one bias op + one output DMA per block.
  - Output bias on device; invalid-action masking on host.
"""

import sys
import numpy as np

sys.path.insert(0, "/opt/trn_rl_repo")

from ml_dtypes import bfloat16

B_FULL = 65536
N_CORES = 8
BC = B_FULL // N_CORES        # 8192 per core
TN = 512                      # batch columns per tile (= matmul free dim)
NT = BC // TN                 # 16 tiles per core
NP = NT // 2                  # 8 pairs (DMA granularity)
TPB = 3                       # tiles per output block (psum rows 0/32/64 only)
NB = (NT + TPB - 1) // TPB    # output blocks
NH, HD, E, HS, A = 4, 3, 12, 8, 20

_CACHE = {}


# ---------------------------------------------------------------- host folding
def _fold_tables(inp):
    f = lambda k: np.asarray(inp[k], np.float64)
    card_emb = f("card_emb")
    in_w, in_b = f("in_w"), f("in_b")
    out_w, out_b = f("out_w"), f("out_b")
    gs_w1, gs_b1, gs_w2, gs_b2 = f("gs_w1"), f("gs_b1"), f("gs_w2"), f("gs_b2")
    dp_w1, dp_b1, dp_w2, dp_b2 = f("dp_w1"), f("dp_b1"), f("dp_w2"), f("dp_b2")
    ctx_w1, ctx_b1, ctx_w2, ctx_b2 = f("ctx_w1"), f("ctx_b1"), f("ctx_w2"), f("ctx_b2")
    sc_w1, sc_b1, sc_w2, sc_b2 = f("sc_w1"), f("sc_b1"), f("sc_w2"), f("sc_b2")
    aci = np.asarray(inp["action_card_indices"])

    Tq = card_emb @ in_w[0:12].T + in_b[0:12]
    Tk = card_emb @ in_w[12:24].T + in_b[12:24]
    Tv = card_emb @ in_w[24:36].T + in_b[24:36]
    G = np.zeros((NH, 54, 54))
    for h in range(NH):
        G[h] = (Tq[:, 3 * h:3 * h + 3] @ Tk[:, 3 * h:3 * h + 3].T) / np.sqrt(HD)
    EG0 = np.exp(G - G.max(axis=2, keepdims=True))
    EG0[:, :, 0] = 0.0

    T = {}

    def den_lhsT(heads):
        out = np.zeros((54, 108))
        for j, h in enumerate(heads):
            out[:, 54 * j:54 * j + 54] = EG0[h].T
        return out

    def w2_lhsT(heads):
        out = np.zeros((108, 108))
        for j, h in enumerate(heads):
            out[54 * j:54 * j + 54, 54 * j:54 * j + 54] = EG0[h]
        return out

    W1hh = ctx_w1[:, 0:12] @ out_w
    u0 = 8.0 * (ctx_w1[:, 0:12] @ out_b)

    def big_lhsT(heads, with_u0):
        out = np.zeros((121 if with_u0 else 108, 128))
        for j, h in enumerate(heads):
            out[54 * j:54 * j + 54, :] = Tv[:, 3 * h:3 * h + 3] @ W1hh[:, 3 * h:3 * h + 3].T
        if with_u0:
            out[108:120, :] = ctx_w1[:, 12:24].T
            out[120, :] = u0
        return out

    T["t_denA"], T["t_denB"] = den_lhsT((0, 1)), den_lhsT((2, 3))
    T["t_denA"] = np.concatenate([T["t_denA"], np.zeros((10, 108))], axis=0)
    T["t_denB"] = np.concatenate([T["t_denB"], np.zeros((10, 108))], axis=0)
    T["t_w2A"], T["t_w2B"] = w2_lhsT((0, 1)), w2_lhsT((2, 3))
    T["t_bigA"] = big_lhsT((0, 1), True)
    T["t_bigB"] = big_lhsT((2, 3), False)

    t_gd = np.zeros((66, 128))
    t_gd[0:12, 0:64] = gs_w1.T
    t_gd[12:66, 64:128] = dp_w1.T
    T["t_gd"] = t_gd
    T["b_gd"] = np.concatenate([gs_b1, dp_b1])[:, None]

    t_agd = np.zeros((128, 128))
    t_agd[0:64, :] = (ctx_w1[:, 24:30] @ gs_w2).T
    t_agd[64:128, :] = (ctx_w1[:, 30:36] @ dp_w2).T
    T["t_agd"] = t_agd

    bias_ctx1 = ctx_b1 + ctx_w1[:, 24:30] @ gs_b2 + ctx_w1[:, 30:36] @ dp_b2
    T["b_ctx1"] = bias_ctx1[:, None]

    W_uc = sc_w1[:, 0:128] @ ctx_w2
    t_uc4 = np.zeros((128, 128))
    for a in range(4):
        t_uc4[:, 32 * a:32 * a + 32] = W_uc.T
    T["t_uc4"] = t_uc4

    am = (aci != 0).astype(np.float64)
    cnt = np.maximum(am.sum(axis=1), 1.0)
    arep = (card_emb[aci] * am[:, :, None]).sum(axis=1) / cnt[:, None]
    v = arep @ sc_w1[:, 128:140].T + sc_b1 + sc_w1[:, 0:128] @ ctx_b2  # [20,32]
    b_H = np.zeros((128, 5))
    for g in range(5):
        for a in range(4):
            b_H[32 * a:32 * a + 32, g] = v[4 * g + a]
    T["b_H"] = b_H

    # t_sc[g]: [128, 32] (12 zero pad cols so the full 32-row psum block is
    # written); score rows j<20 map action 4g+a from H block a
    for g in range(5):
        t = np.zeros((128, 32))
        for a in range(4):
            t[32 * a:32 * a + 32, 4 * g + a] = sc_w2[0]
        T[f"t_sc{g}"] = t

    b_out = np.zeros((128, 1))
    for pp in range(TPB):
        b_out[32 * pp:32 * pp + 20, 0] = float(np.asarray(sc_b2).reshape(-1)[0])
    T["b_out128"] = b_out
    return T


# bf16 weight blob layout: name -> (rows, cols)
W16_LAYOUT = [
    ("t_gd", 66, 128), ("t_agd", 128, 128), ("t_uc4", 128, 128),
    ("t_bigA", 121, 128), ("t_bigB", 108, 128),
    ("t_sc0", 128, 32), ("t_sc1", 128, 32), ("t_sc2", 128, 32),
    ("t_sc3", 128, 32), ("t_sc4", 128, 32), ("t_zero", 32, 128),
    ("t_denA", 64, 108), ("t_denB", 64, 108),
]
W16_COLS = sum(c for _, _, c in W16_LAYOUT)
# f32r weight blob (attention w2 chain; rhs comes from the f32 gpsimd T-mult)
W32_LAYOUT = [
    ("t_w2A", 108, 108), ("t_w2B", 108, 108),
]
W32_COLS = sum(c for _, _, c in W32_LAYOUT)
BIAS_LAYOUT = [("b_gd", 128, 1), ("b_ctx1", 128, 1), ("b_H", 128, 5),
               ("b_out128", 128, 1)]
BIAS_COLS = sum(c for _, _, c in BIAS_LAYOUT)


def _pack_blobs(T):
    T = dict(T)
    T["t_zero"] = np.zeros((32, 128))
    w16 = np.zeros((128, W16_COLS), np.float32)
    off = 0
    for name, rows, cols in W16_LAYOUT:
        w16[0:rows, off:off + cols] = T[name]
        off += cols
    w32 = np.zeros((128, W32_COLS), np.float32)
    off = 0
    for name, rows, cols in W32_LAYOUT:
        w32[0:rows, off:off + cols] = T[name]
        off += cols
    bb = np.zeros((128, BIAS_COLS), np.float32)
    off = 0
    for name, rows, cols in BIAS_LAYOUT:
        bb[0:rows, off:off + cols] = T[name]
        off += cols
    return w16.astype(bfloat16), np.ascontiguousarray(w32), np.ascontiguousarray(bb)


# ---------------------------------------------------------------- bass module
def _build_module(bc):
    import concourse.bass as bass
    import concourse.bacc as bacc
    import concourse.mybir as mybir
    from concourse import tile

    dt = mybir.dt
    f32, f32r, bf16 = dt.float32, dt.float32r, dt.bfloat16
    nt = bc // TN
    npair = nt // 2
    nblk = (nt + TPB - 1) // TPB
    Relu = mybir.ActivationFunctionType.Relu
    Ident = mybir.ActivationFunctionType.Identity
    Copy = mybir.ActivationFunctionType.Copy
    mult = mybir.AluOpType.mult
    add = mybir.AluOpType.add
    amax = mybir.AluOpType.max

    nc = bacc.Bacc("TRN2", target_bir_lowering=False, debug=False)

    din = lambda name, shape, dtype: nc.dram_tensor(name, list(shape), dtype, kind="ExternalInput").ap()
    w16_d = din("w16", (128, W16_COLS), bf16)
    w32_d = din("w32", (128, W32_COLS), f32r)
    bb_d = din("bblob", (128, BIAS_COLS), f32)
    # nsc ships duplicated per tile ([t|t]) so the T-mult is one gpsimd op
    nsc_d = din("nsc", (nt, 108, 2 * TN), f32r)
    nscb_d = din("nscb", (nt, 64, TN), bf16)
    x66_d = din("x66", (npair, 66, 2 * TN), bf16)
    exu_d = din("exu", (npair, 13, 2 * TN), bf16)
    out_d = nc.dram_tensor("outb", [nblk, 128, TN], f32, kind="ExternalOutput").ap()

    with tile.TileContext(nc) as tc:
        with (
            tc.tile_pool(name="const", bufs=1) as cpool,
            tc.tile_pool(name="io", bufs=3) as io,
            tc.tile_pool(name="work", bufs=2) as wk,
            tc.tile_pool(name="ps", bufs=1, space="PSUM") as ps,
        ):
            w16b = cpool.tile([128, W16_COLS], bf16, name="w16b")
            nc.scalar.dma_start(out=w16b, in_=w16_d)
            w32b = cpool.tile([128, W32_COLS], f32r, name="w32b")
            nc.scalar.dma_start(out=w32b, in_=w32_d)
            bblob = cpool.tile([128, BIAS_COLS], f32, name="bblob")
            nc.scalar.dma_start(out=bblob, in_=bb_d)
            tb = {}
            off = 0
            for name, rows, cols in W16_LAYOUT:
                tb[name] = w16b[0:rows, off:off + cols]
                off += cols
            off = 0
            for name, rows, cols in W32_LAYOUT:
                tb[name] = w32b[0:rows, off:off + cols]
                off += cols
            off = 0
            for name, rows, cols in BIAS_LAYOUT:
                tb[name] = bblob[0:rows, off:off + cols]
                off += cols

            # software-pipelined emission: stage A (den/recip/T) runs one tile
            # ahead of stage B (w2 onward) so the PE always has queued work
            # while the DVE chain recip->T of the current tile completes.
            P = {}   # per-pair state
            S = {}   # per-tile state
            SC = {}  # per-block psum
            bsz = [TPB] * (nt // TPB) + ([nt % TPB] if nt % TPB else [])
            if len(bsz) >= 2 and bsz[-1] == 1:
                bsz[-2:] = [2, 2]   # avoid a lone trailing tile
            BLK = [(b, pp, pp == s - 1)
                   for b, s in enumerate(bsz) for pp in range(s)]

            def stageA(t):
                p, ti = divmod(t, 2)
                nscb = io.tile([64, TN], bf16, tag="nscb", bufs=4,
                               name=f"nscb_{t}")
                nc.sync.dma_start(out=nscb, in_=nscb_d[t])
                nsct = io.tile([108, 2 * TN], f32r, tag="nsc", bufs=4,
                               name=f"nsc_{t}")
                nc.sync.dma_start(out=nsct, in_=nsc_d[t])
                nscf = nsct.bitcast(f32)
                if ti == 0:
                    st = {}
                    st["x"] = io.tile([66, 2 * TN], bf16, tag="x", name=f"x_{p}")
                    nc.sync.dma_start(out=st["x"], in_=x66_d[p])
                    st["YA"] = wk.tile([121, 2 * TN], bf16, tag="YA",
                                       name=f"YA_{p}")
                    nc.gpsimd.dma_start(out=st["YA"][108:121, :], in_=exu_d[p])
                    st["YB"] = wk.tile([108, 2 * TN], bf16, tag="YB",
                                       name=f"YB_{p}")
                    st["u4s"] = wk.tile([128, 2 * TN], bf16, tag="u4s",
                                        name=f"u4s_{p}")
                    st["H"] = [wk.tile([128, 2 * TN], bf16, tag=f"H{g}",
                                       name=f"H{g}_{p}") for g in range(5)]
                    P[p] = st
                st = P[p]

                den = ps.tile([108, 2 * TN], f32, tag="att", bufs=2, name=f"den_{t}")
                nc.tensor.matmul(den[:, 0:TN], tb["t_denA"], nscb,
                                 start=True, stop=True)
                nc.tensor.matmul(den[:, TN:2 * TN], tb["t_denB"], nscb,
                                 start=True, stop=True)
                rd = wk.tile([108, 2 * TN], f32, tag="rd", bufs=3, name=f"rd_{t}")
                nc.vector.reciprocal_approx_fast(out=rd, in_=den)
                Tt = wk.tile([108, 2 * TN], f32r, tag="T", bufs=3, name=f"T_{t}")
                if t >= nt - 3:
                    nc.vector.tensor_tensor(Tt, nscf, rd, mult)
                else:
                    nc.gpsimd.tensor_tensor(Tt, nscf, rd, mult)
                S[t] = (Tt, nsct)

            def stageB(t):
                p, ti = divmod(t, 2)
                st = P[p]
                Tt, nsct = S.pop(t)
                nscf = nsct.bitcast(f32)
                one = slice(0, TN)  # noqa: F841
                half = slice(TN * ti, TN * ti + TN)

                w2 = ps.tile([108, 2 * TN], f32, tag="att", bufs=2, name=f"w2_{t}")
                nc.tensor.matmul(w2[:, 0:TN], tb["t_w2A"], Tt[:, 0:TN],
                                 start=True, stop=True)
                nc.tensor.matmul(w2[:, TN:2 * TN], tb["t_w2B"], Tt[:, TN:2 * TN],
                                 start=True, stop=True)
                nc.vector.tensor_tensor(st["YA"][0:108, half], w2[:, 0:TN],
                                        nscf[:, one], mult)
                nc.vector.tensor_tensor(st["YB"][:, half], w2[:, TN:2 * TN],
                                        nscf[:, one], mult)

                gd1 = ps.tile([128, TN], f32, tag="g1", bufs=2, name=f"gd1_{t}")
                nc.tensor.matmul(gd1, tb["t_gd"], st["x"][:, half],
                                 start=True, stop=True)
                gd1r = wk.tile([128, TN], bf16, tag="gd1r", bufs=3, name=f"gd1r_{t}")
                if t >= nt - 3:
                    nc.vector.tensor_scalar(gd1r, gd1, tb["b_gd"], 0.0, add, amax)
                else:
                    nc.scalar.activation(gd1r, gd1, Relu, bias=tb["b_gd"])

                ctx1p = ps.tile([128, TN], f32, tag="g1", bufs=2, name=f"c1_{t}")
                nc.tensor.matmul(ctx1p, tb["t_bigA"], st["YA"][:, half],
                                 start=True, stop=False)
                nc.tensor.matmul(ctx1p, tb["t_bigB"], st["YB"][:, half],
                                 start=False, stop=False)
                nc.tensor.matmul(ctx1p, tb["t_agd"], gd1r, start=False, stop=True)
                ctx1 = wk.tile([128, TN], bf16, tag="ctx1", bufs=3, name=f"ctx1_{t}")
                if t >= nt - 3:
                    nc.vector.tensor_scalar(ctx1, ctx1p, tb["b_ctx1"], 0.0, add, amax)
                else:
                    nc.scalar.activation(ctx1, ctx1p, Relu, bias=tb["b_ctx1"])

                u4 = ps.tile([128, TN], f32, tag="g1", bufs=2, name=f"u4_{t}")
                nc.tensor.matmul(u4, tb["t_uc4"], ctx1, start=True, stop=True)
                nc.scalar.activation(st["u4s"][:, half], u4, Copy)

            def stageC(t):
                p, ti = divmod(t, 2)
                st = P[p]
                u4s = st["u4s"]
                half = slice(TN * ti, TN * ti + TN)
                for g in range(5):
                    if g < 3 or t >= nt - 3:
                        nc.vector.tensor_scalar(st["H"][g][:, half], u4s[:, half],
                                                tb["b_H"][:, g:g + 1], 0.0, add, amax)
                    else:
                        nc.scalar.activation(st["H"][g][:, half], u4s[:, half],
                                             Relu, bias=tb["b_H"][:, g:g + 1])
                b, pp, last = BLK[t]
                if pp == 0:
                    scp = ps.tile([128, TN], f32, tag="sc4", bufs=2,
                                  name=f"sc4_{b}")
                    # zero-weight matmul: clears the bank (start=True) and
                    # writes zeros so the per-tile groups below can
                    # pure-accumulate onto disjoint 32-row blocks.
                    nc.tensor.matmul(scp, tb["t_zero"], u4s[0:32, 0:TN],
                                     start=True, stop=False,
                                     skip_group_check=True)
                    SC[b] = scp
                scp = SC[b]
                for g in range(5):
                    nc.tensor.matmul(scp[32 * pp:32 * pp + 32, :],
                                     tb[f"t_sc{g}"], st["H"][g][:, half],
                                     start=False, stop=(last and g == 4),
                                     skip_group_check=True)
                if last:
                    outs = wk.tile([128, TN], f32, tag="outs",
                                   name=f"outs_{b}")
                    nc.scalar.activation(outs, SC.pop(b), Ident,
                                         bias=tb["b_out128"])
                    nc.sync.dma_start(out=out_d[b], in_=outs)

            stageA(0)
            stageA(1)
            for t in range(nt):
                stageB(t)
                # A before C: den(t+2) is data-ready immediately, while the
                # score matmuls in C(t) wait on the H-relu latency -- keep
                # the in-order PE fed during that window.
                if t + 2 < nt:
                    stageA(t + 2)
                stageC(t)

    nc.finalize()
    return nc


def _get_module(bc=BC):
    key = ("mod", bc)
    if key not in _CACHE:
        _CACHE[key] = _build_module(bc)
    return _CACHE[key]


# ---------------------------------------------------------------- host prep
def _prep_data(inp):
    """Full-batch host prep: counts, scaling, layout. Returns per-core input maps."""
    hc = np.asarray(inp["hand_cards"])
    B = hc.shape[0]
    gs = np.asarray(inp["game_state"], np.float32)
    dp = np.asarray(inp["discard_pile_cards"], np.float32)
    en = np.asarray(inp["enemy_card"]).reshape(B).astype(np.int64)
    hsz = np.asarray(inp["hand_size"]).astype(np.float64)

    idx = (hc.astype(np.int64) + 54 * np.arange(B, dtype=np.int64)[:, None]).ravel()
    counts = np.bincount(idx, minlength=B * 54).reshape(B, 54)
    rlen = (1.0 / np.maximum(hsz, 1.0)).astype(np.float32)
    nsc54 = (counts.astype(np.float32) * rlen[:, None]).T          # [54, B]
    nsc = np.concatenate([nsc54, nsc54], axis=0)                   # [108, B]
    nsc_t = np.broadcast_to(
        nsc.reshape(108, B // TN, 1, TN), (108, B // TN, 2, TN))
    nsc_t = np.ascontiguousarray(nsc_t.transpose(1, 0, 2, 3)).reshape(
        B // TN, 108, 2 * TN)                                      # [NTtot,108,2TN]

    x66 = np.empty((66, B), np.float32)
    x66[0:12] = gs.T
    x66[12:66] = dp.T
    x66 = x66.astype(bfloat16)
    en_emb = np.asarray(inp["enemy_emb"], np.float32)
    exu = np.empty((13, B), np.float32)
    exu[0:12] = en_emb[en].T
    exu[12] = rlen
    exu = exu.astype(bfloat16)

    tables = _fold_tables(inp)
    w16, w32, bb = _pack_blobs(tables)

    def pairize(arr, rows):
        # [rows, BC] -> [NP, rows, 2*TN]
        return np.ascontiguousarray(
            arr.reshape(rows, NP, 2 * TN).transpose(1, 0, 2))

    maps = []
    for c in range(N_CORES):
        cols = slice(c * BC, (c + 1) * BC)
        tlo = c * NT
        nscb_c = np.concatenate(
            [nsc54[:, cols], np.zeros((10, BC), np.float32)],
            axis=0).reshape(64, NT, TN).transpose(1, 0, 2)
        m = {"w16": w16, "w32": w32, "bblob": bb,
             "nsc": np.ascontiguousarray(nsc_t[tlo:tlo + NT]),
             "nscb": np.ascontiguousarray(nscb_c).astype(bfloat16),
             "x66": pairize(np.ascontiguousarray(x66[:, cols]), 66),
             "exu": pairize(np.ascontiguousarray(exu[:, cols]), 13)}
        maps.append(m)
    return maps


# ---------------------------------------------------------------- entry points
def _enable_ldw_opt():
    # Dedup/pipeline PE weight loads.
    import concourse.bass_utils as bu
    if getattr(bu, "_ldw_opt_patched", False):
        return
    orig = bu.run_command

    def patched(argv, **kw):
        argv = [a.replace("--enable-ldw-opt=false", "--enable-ldw-opt=true")
                if isinstance(a, str) else a for a in argv]
        return orig(argv, **kw)

    bu.run_command = patched
    bu._ldw_opt_patched = True


def _unpack_output(results, inp):
    """outb [NB, 128, TN] per core -> full [B, 20] with invalid-action mask."""
    nva = int(inp["num_valid_actions"])
    bsz = [TPB] * (NT // TPB) + ([NT % TPB] if NT % TPB else [])
    if len(bsz) >= 2 and bsz[-1] == 1:
        bsz[-2:] = [2, 2]
    blk = [(b, pp) for b, s in enumerate(bsz) for pp in range(s)]
    out = np.empty((B_FULL, A), np.float32)
    for c, r in enumerate(results):
        ob = r["outb"]                                   # [NB, 128, TN]
        for t in range(NT):
            b, pp = blk[t]
            colbase = c * BC + t * TN
            out[colbase:colbase + TN, :] = ob[b, 32 * pp:32 * pp + A, :].T
    if nva < A:
        out[:, nva:] = -1e8
    return out


def _run(inputs, trace=False):
    from concourse.bass_utils import run_bass_kernel_spmd
    # NOTE: walrus --enable-ldw-opt rejects the standalone InstLdweights that
    # bf16 matmuls lower to; the PE's 64-deep reorder window pulls those
    # LDWEIGHTS ahead in silicon, so the opt pass isn't needed here.

    in_maps = _prep_data(inputs)
    nc = _get_module()
    res = run_bass_kernel_spmd(nc, in_maps, list(range(N_CORES)), trace=trace)
    out = _unpack_output(res.results, inputs)
    return out, res


def kernel(**inputs) -> np.ndarray:
    try:
        out, _ = _run(inputs, trace=False)
    except Exception:
        # one retry: transient NRT device errors have been observed to clear
        # on re-execution
        out, _ = _run(inputs, trace=False)
    return out
